# revision 2
# baseline (speedup 1.0000x reference)
"""Trainium2 Bass kernel v3 for nn_CustomLoss_51677046505531.

loss = 0.5 * mean((logits-labels)^2)
     + 0.5 * sum_{labels_i > labels_j} relu(1 - (logits_i - logits_j)) / #pairs

Host sorts by labels; device sums relu(T_c - g_r) (T = 1+g) over strict-lower
positions (r > c); ties corrected on host.

v3: contiguous-block decomposition to MINIMIZE INSTRUCTION COUNT (HW
per-instruction cost ~0.5-1us dominates the older many-small-jobs designs).

Core c owns row-blocks {8d + sigma_d(c): d=0..7}, sigma_d(c) = c (d even)
or 7-c (d odd) -> per-core total work identical AND per-band job shapes
identical across cores (SPMD-uniform):
  * band job d=1..7 (DVE): rows = block b = 8d+sigma, cols = [0, 1024d)
    of the GLOBAL label-sorted order. One tensor_scalar per band:
    acc[p] = sum_c max(T16_c, g_p) at bf16 4x with fused add-accum;
    host subtracts 1024d * sum_p g_p  (relu(T-g) = max(T,g) - g).
  * leftover cols [1024d, 128b) and the 8 diag-block triangles are host
    premasked/packed dense into ONE [128, FP] bf16 tile; one DVE
    max(v,0)+add-accum instruction (pad = -1).
  * optional ACT offload (act_bands): whole band jobs on ScalarE reading
    the same SBUF bf16 T-broadcast: Relu(in + (-g_p)) with fused accum.
  * MSE partials: 2 small DVE fp32 instructions.
Broadcast: one [128, 7168] bf16 T tile via chunked stride-0 DMA per rep.
"""

import sys

sys.path.insert(0, "/opt/trn_rl_repo")

from contextlib import ExitStack

import ml_dtypes
import numpy as np

import concourse.bass as bass
import concourse.tile as tile
from concourse import mybir
from concourse.bacc import Bacc
from concourse.bass_utils import run_bass_kernel_spmd

ALPHA = 0.5
N = 8192
NCORES = 8
P = 128
B = N // P            # 64 blocks of 128
NB = 8                # bands; band d = blocks [8d, 8d+8)
W = 1024              # cols per band
F32 = mybir.dt.float32
BF16 = mybir.dt.bfloat16
BF = ml_dtypes.bfloat16

ACT_BANDS = (4, 6, 7)  # bands offloaded to ScalarE (concurrent with DVE)

# packed tile: 8 diag triangles + partial-band rectangles (sum sigma = 28)
_PACK_ELEMS = 8 * (P * (P - 1) // 2) + P * P * 28
FP = (_PACK_ELEMS + P - 1) // P   # 4092
COL_PACK = 8
COL_MSE = 9
NACC = 16

_CACHE = {}


def _sigma(d, c):
    return c if d % 2 == 0 else 7 - c


def _blocks(c):
    return [8 * d + _sigma(d, c) for d in range(NB)]


def _build_nc(reps=1, act_bands=ACT_BANDS, skip_dve=False, skip_packed=False,
              skip_bcast=False, skip_mse=False, preload=False):
    nc = Bacc()
    gb = nc.declare_dram_parameter("gb", [1, 7 * W], BF16, isOutput=False)
    grows = nc.declare_dram_parameter("g_rows", [P, NB], F32, isOutput=False)
    negg = nc.declare_dram_parameter("neg_g", [P, NB], F32, isOutput=False)
    packed = nc.declare_dram_parameter("packed", [P, FP], BF16, isOutput=False)
    mse_x = nc.declare_dram_parameter("mse_x", [P, N // NCORES // P], F32, isOutput=False)
    mse_y = nc.declare_dram_parameter("mse_y", [P, N // NCORES // P], F32, isOutput=False)
    out_acc = nc.declare_dram_parameter("out_acc", [P, NACC], F32, isOutput=True)

    relu = mybir.ActivationFunctionType.Relu
    alu = mybir.AluOpType

    with ExitStack() as ctx:
        tc = ctx.enter_context(tile.TileContext(nc))
        const = ctx.enter_context(tc.tile_pool(name="const", bufs=1))

        grows_s = const.tile([P, NB], F32)
        negg_s = const.tile([P, NB], F32)
        msex_s = const.tile([P, N // NCORES // P], F32)
        msey_s = const.tile([P, N // NCORES // P], F32)
        nc.sync.dma_start(out=grows_s, in_=grows[:, :])
        nc.sync.dma_start(out=negg_s, in_=negg[:, :])
        nc.sync.dma_start(out=msex_s, in_=mse_x[:, :])
        nc.sync.dma_start(out=msey_s, in_=mse_y[:, :])

        tb = const.tile([P, 7 * W], BF16)
        packed_s = const.tile([P, FP], BF16)
        acc_s = const.tile([P, NACC], F32)
        dve_scr = const.tile([P, 7 * W], BF16)
        act_scr = const.tile([P, 7 * W], BF16)
        pk_scr = const.tile([P, FP], BF16)
        nmse = N // NCORES // P
        dif = const.tile([P, nmse], F32)
        sqo = const.tile([P, nmse], F32)
        nc.vector.memset(acc_s, 0.0)

        chunks = [(0, W), (W, 2 * W), (3 * W, 2 * W), (5 * W, 2 * W)]

        if preload:
            for off, w in chunks:
                nc.sync.dma_start(
                    out=tb[:, off : off + w],
                    in_=gb[:, off : off + w].to_broadcast([P, w]),
                )
            nc.sync.dma_start(out=packed_s, in_=packed[:, :])

        def emit_compute():
            # T broadcast (global order) into SBUF bf16, chunked ascending
            for off, w in chunks if not (skip_bcast or preload) else []:
                nc.sync.dma_start(
                    out=tb[:, off : off + w],
                    in_=gb[:, off : off + w].to_broadcast([P, w]),
                )
            if not (skip_packed or preload):
                nc.sync.dma_start(out=packed_s, in_=packed[:, :])

            # MSE partials first (inputs resident; frees DVE for band jobs)
            if not skip_mse:
                nc.vector.tensor_sub(dif, msex_s, msey_s)
                nc.vector.scalar_tensor_tensor(
                    out=sqo, in0=dif, scalar=0.0, in1=dif,
                    op0=alu.bypass, op1=alu.mult,
                    accum_out=acc_s[:, COL_MSE : COL_MSE + 1],
                )

            # band jobs
            for d in range(1, NB) if not skip_dve else []:
                if d in act_bands:
                    nc.scalar.activation(
                        out=act_scr[:, : d * W],
                        in_=tb[:, : d * W],
                        func=relu,
                        bias=negg_s[:, d : d + 1],
                        scale=1.0,
                        accum_out=acc_s[:, d : d + 1],
                    )
                else:
                    nc.vector.tensor_scalar(
                        out=dve_scr[:, : d * W],
                        in0=tb[:, : d * W],
                        scalar1=grows_s[:, d : d + 1],
                        scalar2=0.0,
                        op0=alu.max,
                        op1=alu.add,
                        accum_out=acc_s[:, d : d + 1],
                    )

            # packed diag + partial-band rectangles
            if not skip_packed:
                nc.vector.tensor_scalar(
                    out=pk_scr,
                    in0=packed_s,
                    scalar1=0.0,
                    scalar2=0.0,
                    op0=alu.max,
                    op1=alu.add,
                    accum_out=acc_s[:, COL_PACK : COL_PACK + 1],
                )

        if reps > 1:
            with tc.For_i(0, reps, 1):
                emit_compute()
        else:
            emit_compute()

        # Stage through ScalarE: program-order on ACT guarantees its fused
        # accumulator writes have landed before the output DMA reads them.
        acc_stage = const.tile([P, NACC], F32)
        nc.scalar.copy(out=acc_stage, in_=acc_s)
        nc.sync.dma_start(out=out_acc[:, :], in_=acc_stage)

    nc.finalize()
    return nc


def _host_prep(logits, labels):
    logits = np.asarray(logits, dtype=np.float32).reshape(N)
    labels = np.asarray(labels, dtype=np.float32).reshape(N)
    order = np.argsort(labels, kind="stable")
    g = np.ascontiguousarray(logits[order]).astype(np.float32)
    labs = labels[order]
    T16 = (1.0 + g).astype(BF)          # device column values
    T16f = T16.astype(np.float64)
    g64 = g.astype(np.float64)

    # Exact #pairs with labels_i > labels_j; tie pairs (which the positional
    # triangle wrongly includes) are removed using the same values the device
    # adds: relu(T16_c - g_r).
    num_pairs = N * (N - 1) // 2
    tie_corr = 0.0
    change = np.nonzero(np.diff(labs))[0] + 1
    starts = np.concatenate([[0], change])
    ends = np.concatenate([change, [N]])
    for a, e in zip(starts, ends):
        m = int(e - a)
        if m > 1:
            num_pairs -= m * (m - 1) // 2
            dmat = T16f[a:e][None, :] - g64[a:e][:, None]   # [r, c]
            tie_corr += float(np.maximum(dmat, 0.0)[np.tril_indices(m, -1)].sum())

    il = np.tril_indices(P, -1)
    in_maps = []
    for c in range(NCORES):
        blocks = _blocks(c)
        grow = np.stack([g[P * b : P * (b + 1)] for b in blocks], axis=1)  # [P, 8]
        pieces = []
        for d, b in enumerate(blocks):
            gg = g[P * b : P * (b + 1)].astype(np.float32)
            # diag triangle: [r, c] = T16_c - g_r, r > c
            pre = T16[P * b : P * (b + 1)].astype(np.float32)[None, :] - gg[:, None]
            pieces.append(pre[il])
            # partial band: cols [1024d, 128b)
            lo = W * d
            if P * b > lo:
                cols = T16[lo : P * b].astype(np.float32)
                pieces.append((cols[None, :] - gg[:, None]).ravel())
        flat = np.concatenate(pieces).astype(np.float32)
        pad = FP * P - flat.size
        flat = np.concatenate([flat, np.full(pad, -1.0, np.float32)])
        in_maps.append(
            {
                "gb": T16[: 7 * W].reshape(1, 7 * W).copy(),
                "g_rows": np.ascontiguousarray(grow),
                "neg_g": np.ascontiguousarray(-grow),
                "packed": np.ascontiguousarray(flat.reshape(P, FP)).astype(BF),
                "mse_x": np.ascontiguousarray(logits[c::NCORES].reshape(P, -1)),
                "mse_y": np.ascontiguousarray(labels[c::NCORES].reshape(P, -1)),
            }
        )
    return in_maps, num_pairs, tie_corr


def _combine(results, num_pairs, tie_corr, in_maps, act_bands=ACT_BANDS):
    rank_dev = 0.0
    sse = 0.0
    for c in range(NCORES):
        oa = results[c]["out_acc"].astype(np.float64)
        grow = np.asarray(in_maps[c]["g_rows"], np.float64)
        for d in range(1, NB):
            if d in act_bands:
                rank_dev += oa[:, d].sum()        # ACT computed relu directly
            else:
                rank_dev += oa[:, d].sum() - W * d * grow[:, d].sum()
        rank_dev += oa[:, COL_PACK].sum()
        sse += oa[:, COL_MSE].sum()
    rank_sum = rank_dev - tie_corr
    mse = sse / N
    ranking = rank_sum / max(num_pairs, 1) if num_pairs > 0 else 0.0
    return np.float32(ALPHA * mse + (1.0 - ALPHA) * ranking)


def kernel(logits, labels, **_unused):
    in_maps, num_pairs, tie_corr = _host_prep(logits, labels)
    if "nc" not in _CACHE:
        _CACHE["nc"] = _build_nc()
    res = run_bass_kernel_spmd(_CACHE["nc"], in_maps, list(range(NCORES)))
    return _combine(res.results, num_pairs, tie_corr, in_maps)
